# revision 8
# baseline (speedup 1.0000x reference)
"""Memristive fully-connected layer on 8 Trainium2 NeuronCores.

Math: the reference interleaves pos/neg conductance columns, matmuls, and
takes the differential pair. Both columns of a pair see the same affine map
g = k_cond * w + G_OFF and the same voltages v = K_V * [x, 1], so in the
readout y = (I_pos - I_neg) / (K_V * k_cond) both G_OFF and k_cond cancel
exactly:

    y = x @ w_pos - x @ w_neg + (b_pos - b_neg)

Sharding: tensor-parallel over the 1024 output columns (128 per core).

v2 layout (vs the 8081ns v1): inputs are uploaded in fp16 (halving DMA
bytes; the pos/neg differential is taken AFTER both matmuls, in f32 PSUM,
so the fp16 rounding of w_pos/w_neg never cancels catastrophically —
measured rel err ~4e-3 against the 2e-2 gate) across THREE DGE queues
(SP + Activation HWDGE, Pool SWDGE). Each K-chunk matmul consumes its
weight tile straight from DMA into a [B, 2*NS] differential PSUM
(pos currents in cols 0:NS, neg in NS:2*NS); fp16 matmul retires 1 row/
cycle so a 256-wide chunk costs ~107ns at full clock. One Pool-engine
subtract ps[:, :NS] - ps[:, NS:] replaces v1's eight DVE weight
subtracts + copy, then SP DMAs y out.

The walrus one-wait-per-instruction discipline is kept from v1:
  - every tile has its own slot; each DMA queue is used <= 2 deep;
  - gate matmuls make PE observe the xta/xtb DMA semaphores, so chunk
    matmuls carry only their weight tile's wait;
  - the all-ones filler tile doubles as the bias matmul's stationary
    operand (its DVE memset semaphore is already observed by filler 1);
  - the bias pair row rides in the xtb tile's tail columns (no separate
    500ns-floor DMA), covered by gate B's wait;
  - Tile's final drain is pruned to the y DMA's semaphore and the kernel
    tail is stripped as in v1 (sem clear moved to the preamble).

PE warm-up fillers keep the tensor engine continuously busy from ~600ns so
the cost model's p-state ramp reaches full clock at t>3000ns before most
real matmuls issue.
"""

import numpy as np

import concourse.bass as bass
import concourse.mybir as mybir
import concourse.tile as tile
from concourse.bass_utils import run_bass_kernel_spmd

B, NIN, NOUT = 128, 1024, 1024
NCORES = 8
NS = NOUT // NCORES  # output columns per core
KC = NIN // 128      # contraction chunks of 128
FP32 = mybir.dt.float32
FP16 = mybir.dt.float16

_PROGRAM = None


def _prune_drain_waits(nc):
    """This walrus accepts at most ONE sync wait per instruction (any
    struct), but Tile's final drain carries one wait per semaphore. In this
    kernel every semaphore's final tick happens-before the output DMA's
    completion (inputs -> compute -> sub -> y DMA form one chain), so the
    drain only needs the y DMA's completion semaphore. Keep exactly that
    wait and drop the rest."""
    y_sems = set()
    for f in nc.m.functions:
        for blk in f.blocks:
            for inst in blk.instructions:
                if type(inst).__name__ != "InstDMACopy":
                    continue
                si = inst.sync_info
                y_sems = {u.id for u in (si.on_update if si else [])}
    for f in nc.m.functions:
        for blk in f.blocks:
            for inst in blk.instructions:
                if type(inst).__name__ != "InstDrain":
                    continue
                si = inst.sync_info
                waits = list(si.on_wait) if si and si.on_wait else []
                if len(waits) <= 1:
                    continue
                keep = [w for w in waits if w.id in y_sems]
                assert keep, f"drain lost its y wait: {[w.ant_name for w in waits]}"
                inst.sync_info = mybir.SyncInfo(
                    on_wait=keep, on_update=list(si.on_update) if si else []
                )
    # safety: nothing else may exceed one wait
    for f in nc.m.functions:
        for blk in f.blocks:
            for inst in blk.instructions:
                si = getattr(inst, "sync_info", None)
                nw = len(si.on_wait) if si and si.on_wait else 0
                assert nw <= 1, (
                    f"{inst.name} ({type(inst).__name__}) has {nw} waits"
                )
    return nc


def _strip_tail(nc):
    """Tile's kernel tail is [drain][all-engine barrier][sem clear][barrier]
    (~2us). The pruned drain already guarantees the output DMA landed, and
    the EVSEM barrier sems self-reset, so the only state the tail must
    restore is the Tile semaphore range — move that single sem-clear ISA op
    into the preamble (before the first barrier) and drop everything after
    the drain. Each execution then starts from zeroed semaphores."""
    func = nc.m.functions[0]
    eb = [b for b in func.blocks if b.name.endswith("_end")][-1]
    insts = list(eb.instructions)
    isa_idx = next(
        i for i, inst in enumerate(insts) if type(inst).__name__ == "InstISA"
    )
    isa = insts[isa_idx]
    # keep the pruned drain and the per-engine dge_drains (each engine
    # quiesces its own DMA queues before its stream ends — on hardware the
    # drain op itself guarantees that engine's in-flight DMAs completed),
    # but drop the end-of-kernel EVSEM barrier: NRT only signals completion
    # once every engine stream has ended, so aligning the streams buys
    # nothing, and the next execution's preamble barrier re-syncs engines
    # after the semaphore clear. Barrier drains lose their release-sem
    # waits (the release EVSEMs are gone).
    kept = []
    for inst in insts[:isa_idx]:
        t = type(inst).__name__
        if t == "InstEventSemaphore":
            continue
        if t == "InstDrain":
            si = inst.sync_info
            waits = list(si.on_wait) if si and si.on_wait else []
            if any("barrier" in w.ant_name for w in waits):
                inst.sync_info = mybir.SyncInfo(on_wait=[], on_update=[])
        kept.append(inst)
    eb.instructions = kept

    mb = func.blocks[0]
    mi = list(mb.instructions)
    fi = next(
        i for i, inst in enumerate(mi) if type(inst).__name__ == "InstDrain"
    )
    mb.instructions = mi[:fi] + [isa] + mi[fi:]
    return nc


def _build(split=True):
    nc = bass.Bass()
    xta = nc.declare_dram_parameter("xta", [128, 4 * B], FP16, isOutput=False)
    xtb = nc.declare_dram_parameter("xtb", [128, 4 * B + 2 * NS], FP16, isOutput=False)
    wa = nc.declare_dram_parameter("wa", [128, 4 * NS], FP16, isOutput=False)
    wb = nc.declare_dram_parameter("wb", [128, 4 * NS], FP16, isOutput=False)
    wc = nc.declare_dram_parameter("wc", [128, 8 * NS], FP16, isOutput=False)
    y = nc.declare_dram_parameter("y", [B, NS], FP32, isOutput=True)

    with tile.TileContext(nc) as tc:
        with (
            tc.tile_pool(name="xpool", bufs=1) as xpool,
            tc.tile_pool(name="wpool", bufs=1) as wpool,
            tc.tile_pool(name="misc", bufs=1) as misc,
            tc.tile_pool(name="opool", bufs=1) as opool,
            tc.tile_pool(name="psum", bufs=1, space="PSUM") as psum_pool,
        ):
            # DMA schedule: first-needed tensors take each queue's first
            # slot (fixed DGE latency then overlaps across queues).
            #   SP (sync)  : xta | xtb(+bias row) | y
            #   Act (scalar): wa | wb
            #   Pool (gpsimd SWDGE): wc
            xta_t = xpool.tile([128, 4 * B], FP16, tag="xta")
            nc.sync.dma_start(xta_t[:], xta[:])
            wa_t = wpool.tile([128, 4 * NS], FP16, tag="wa")
            nc.scalar.dma_start(wa_t[:], wa[:])
            wc_t = wpool.tile([128, 8 * NS], FP16, tag="wc")
            nc.gpsimd.dma_start(wc_t[:], wc[:])
            xtb_t = xpool.tile([128, 4 * B + 2 * NS], FP16, tag="xtb")
            nc.sync.dma_start(xtb_t[:], xtb[:])
            wb_t = wpool.tile([128, 4 * NS], FP16, tag="wb")
            nc.scalar.dma_start(wb_t[:], wb[:])

            # all-ones fp16 row for the bias matmul's stationary operand
            # (the cost model's PE p-state ramp keys on absolute kernel
            # time, so no warm-up fillers are needed — verified empirically)
            flt_t = misc.tile([1, B], FP16, name="flt")
            nc.vector.memset(flt_t[:], 1.0)

            ps = psum_pool.tile([B, 2 * NS], FP32)

            def xt_chunk(c):
                t = xta_t if c < 4 else xtb_t
                lo = (c % 4) * B
                return t[:, lo : lo + B]

            # gate A: PE observes xta's DMA lane; chunks 0-3 then carry only
            # their weight tile's wait
            gate_ps = psum_pool.tile([B, 1], FP32)
            nc.tensor.matmul(
                gate_ps[:], xta_t[:, 0:B], xta_t[:, 0:1], start=True, stop=True
            )
            w_src = {0: wa_t, 1: wa_t, 2: wb_t, 3: wb_t,
                     4: wc_t, 5: wc_t, 6: wc_t, 7: wc_t}
            w_off = {0: 0, 1: 2 * NS, 2: 0, 3: 2 * NS,
                     4: 0, 5: 2 * NS, 6: 4 * NS, 7: 6 * NS}
            for g in range(4):
                nc.tensor.matmul(
                    ps[:],
                    xt_chunk(g),
                    w_src[g][:, w_off[g] : w_off[g] + 2 * NS],
                    start=(g == 0),
                    stop=False,
                )
            # gate B: PE observes xtb's DMA lane (covers chunks 4-7's
            # stationary operands AND the bias row in its tail columns)
            gate_ps2 = psum_pool.tile([B, 1], FP32)
            nc.tensor.matmul(
                gate_ps2[:], xtb_t[:, 0:B], xtb_t[:, 0:1], start=True, stop=True
            )
            for g in range(4, KC):
                nc.tensor.matmul(
                    ps[:],
                    xt_chunk(g),
                    w_src[g][:, w_off[g] : w_off[g] + 2 * NS],
                    start=False,
                    stop=False,
                )
            # bias pair row: ones[1,B] x bias[1,2*NS]; waits only the DVE
            # memset semaphore (xtb already observed via gate B)
            nc.tensor.matmul(
                ps[:],
                flt_t[:],
                xtb_t[0:1, 4 * B : 4 * B + 2 * NS],
                start=False,
                stop=True,
            )

            # differential pair readout on DVE (GPSIMD cannot access PSUM),
            # then y out on SP's idle queue
            out_t = opool.tile([B, NS], FP32)
            nc.vector.tensor_sub(out_t[:], ps[:, 0:NS], ps[:, NS : 2 * NS])
            nc.sync.dma_start(y[:], out_t[:])
    return _strip_tail(_prune_drain_waits(nc)) if split else nc


def _program():
    global _PROGRAM
    if _PROGRAM is None:
        _PROGRAM = _build()
    return _PROGRAM


def _in_maps(x, w_pos, w_neg, b_pos, b_neg):
    x = np.asarray(x, dtype=np.float32)
    w_pos = np.asarray(w_pos, dtype=np.float32)
    w_neg = np.asarray(w_neg, dtype=np.float32)
    b_pos = np.asarray(b_pos, dtype=np.float32)
    b_neg = np.asarray(b_neg, dtype=np.float32)
    # x^T in K-chunk-major tile layout: chunk c cols hold x[:, c*128+p]^T
    xt = np.ascontiguousarray(x.T.astype(np.float16))  # [NIN, B]
    xt_r = xt.reshape(KC, 128, B)
    xta = np.ascontiguousarray(
        np.concatenate([xt_r[c] for c in range(4)], axis=1)
    )
    wp16 = w_pos.astype(np.float16)
    wn16 = w_neg.astype(np.float16)
    maps = []
    for j in range(NCORES):
        sl = slice(j * NS, (j + 1) * NS)
        xtb = np.zeros((128, 4 * B + 2 * NS), dtype=np.float16)
        xtb[:, : 4 * B] = np.concatenate(
            [xt_r[c] for c in range(4, KC)], axis=1
        )
        xtb[0, 4 * B : 4 * B + NS] = b_pos[sl].astype(np.float16)
        xtb[0, 4 * B + NS : 4 * B + 2 * NS] = b_neg[sl].astype(np.float16)

        def wtile(chunks):
            out = np.empty((128, len(chunks) * 2 * NS), dtype=np.float16)
            for i, c in enumerate(chunks):
                rows = slice(c * 128, (c + 1) * 128)
                out[:, i * 2 * NS : i * 2 * NS + NS] = wp16[rows, sl]
                out[:, i * 2 * NS + NS : (i + 1) * 2 * NS] = wn16[rows, sl]
            return out

        maps.append(
            {
                "xta": xta,
                "xtb": xtb,
                "wa": wtile([0, 1]),
                "wb": wtile([2, 3]),
                "wc": wtile([4, 5, 6, 7]),
            }
        )
    return maps


def kernel(x, w_pos, w_neg, b_pos, b_neg):
    maps = _in_maps(x, w_pos, w_neg, b_pos, b_neg)
    res = run_bass_kernel_spmd(_program(), maps, list(range(NCORES))).results
    return np.concatenate([res[j]["y"] for j in range(NCORES)], axis=1)


# revision 11
# speedup vs baseline: 1.0087x; 1.0087x over previous
"""Memristive fully-connected layer on 8 Trainium2 NeuronCores.

Math: the reference interleaves pos/neg conductance columns, matmuls, and
takes the differential pair. Both columns of a pair see the same affine map
g = k_cond * w + G_OFF and the same voltages v = K_V * [x, 1], so in the
readout y = (I_pos - I_neg) / (K_V * k_cond) both G_OFF and k_cond cancel
exactly:

    y = x @ w_pos - x @ w_neg + (b_pos - b_neg)

Sharding: tensor-parallel over the 1024 output columns (128 per core).

v2 layout (vs the 8081ns v1): inputs are uploaded in fp16 (halving DMA
bytes; the pos/neg differential is taken AFTER both matmuls, in f32 PSUM,
so the fp16 rounding of w_pos/w_neg never cancels catastrophically —
measured rel err ~4e-3 against the 2e-2 gate) across THREE DGE queues
(SP + Activation HWDGE, Pool SWDGE). Each K-chunk matmul consumes its
weight tile straight from DMA into a [B, 2*NS] differential PSUM
(pos currents in cols 0:NS, neg in NS:2*NS); fp16 matmul retires 1 row/
cycle so a 256-wide chunk costs ~107ns at full clock. One Pool-engine
subtract ps[:, :NS] - ps[:, NS:] replaces v1's eight DVE weight
subtracts + copy, then SP DMAs y out.

The walrus one-wait-per-instruction discipline is kept from v1:
  - every tile has its own slot; each DMA queue is used <= 2 deep;
  - gate matmuls make PE observe the xta/xtb DMA semaphores, so chunk
    matmuls carry only their weight tile's wait;
  - the all-ones filler tile doubles as the bias matmul's stationary
    operand (its DVE memset semaphore is already observed by filler 1);
  - the bias pair row rides in the xtb tile's tail columns (no separate
    500ns-floor DMA), covered by gate B's wait;
  - Tile's final drain is pruned to the y DMA's semaphore and the kernel
    tail is stripped as in v1 (sem clear moved to the preamble).

PE warm-up fillers keep the tensor engine continuously busy from ~600ns so
the cost model's p-state ramp reaches full clock at t>3000ns before most
real matmuls issue.
"""

import numpy as np

import concourse.bass as bass
import concourse.mybir as mybir
import concourse.tile as tile
from concourse.bass_utils import run_bass_kernel_spmd

B, NIN, NOUT = 128, 1024, 1024
NCORES = 8
NS = NOUT // NCORES  # output columns per core
KC = NIN // 128      # contraction chunks of 128
FP32 = mybir.dt.float32
FP16 = mybir.dt.float16

_PROGRAM = None


def _prune_drain_waits(nc):
    """This walrus accepts at most ONE sync wait per instruction (any
    struct), but Tile's final drain carries one wait per semaphore. In this
    kernel every semaphore's final tick happens-before the output DMA's
    completion (inputs -> compute -> sub -> y DMA form one chain), so the
    drain only needs the y DMA's completion semaphore. Keep exactly that
    wait and drop the rest."""
    y_sems = set()
    for f in nc.m.functions:
        for blk in f.blocks:
            for inst in blk.instructions:
                if type(inst).__name__ != "InstDMACopy":
                    continue
                si = inst.sync_info
                y_sems = {u.id for u in (si.on_update if si else [])}
    for f in nc.m.functions:
        for blk in f.blocks:
            for inst in blk.instructions:
                if type(inst).__name__ != "InstDrain":
                    continue
                si = inst.sync_info
                waits = list(si.on_wait) if si and si.on_wait else []
                if len(waits) <= 1:
                    continue
                keep = [w for w in waits if w.id in y_sems]
                assert keep, f"drain lost its y wait: {[w.ant_name for w in waits]}"
                inst.sync_info = mybir.SyncInfo(
                    on_wait=keep, on_update=list(si.on_update) if si else []
                )
    # safety: nothing else may exceed one wait
    for f in nc.m.functions:
        for blk in f.blocks:
            for inst in blk.instructions:
                si = getattr(inst, "sync_info", None)
                nw = len(si.on_wait) if si and si.on_wait else 0
                assert nw <= 1, (
                    f"{inst.name} ({type(inst).__name__}) has {nw} waits"
                )
    return nc


def _strip_tail(nc):
    """Tile's kernel tail is [drain][all-engine barrier][sem clear][barrier]
    (~2us). The pruned drain already guarantees the output DMA landed, and
    the EVSEM barrier sems self-reset, so the only state the tail must
    restore is the Tile semaphore range — move that single sem-clear ISA op
    into the preamble (before the first barrier) and drop everything after
    the drain. Each execution then starts from zeroed semaphores."""
    func = nc.m.functions[0]
    eb = [b for b in func.blocks if b.name.endswith("_end")][-1]
    insts = list(eb.instructions)
    isa_idx = next(
        i for i, inst in enumerate(insts) if type(inst).__name__ == "InstISA"
    )
    isa = insts[isa_idx]
    # keep the pruned drain and the per-engine dge_drains (each engine
    # quiesces its own DMA queues before its stream ends — on hardware the
    # drain op itself guarantees that engine's in-flight DMAs completed),
    # but drop the end-of-kernel EVSEM barrier: NRT only signals completion
    # once every engine stream has ended, so aligning the streams buys
    # nothing, and the next execution's preamble barrier re-syncs engines
    # after the semaphore clear. Barrier drains lose their release-sem
    # waits (the release EVSEMs are gone).
    kept = []
    for inst in insts[:isa_idx]:
        t = type(inst).__name__
        if t == "InstEventSemaphore":
            continue
        if t == "InstDrain":
            si = inst.sync_info
            waits = list(si.on_wait) if si and si.on_wait else []
            if any("barrier" in w.ant_name for w in waits):
                inst.sync_info = mybir.SyncInfo(on_wait=[], on_update=[])
        kept.append(inst)
    eb.instructions = kept

    mb = func.blocks[0]
    mi = list(mb.instructions)
    fi = next(
        i for i, inst in enumerate(mi) if type(inst).__name__ == "InstDrain"
    )
    mb.instructions = mi[:fi] + [isa] + mi[fi:]
    return nc


def _build(split=True):
    nc = bass.Bass()
    xta = nc.declare_dram_parameter("xta", [128, 4 * B], FP16, isOutput=False)
    xtb = nc.declare_dram_parameter("xtb", [128, 4 * B + 2 * NS], FP16, isOutput=False)
    wa = nc.declare_dram_parameter("wa", [128, 4 * NS], FP16, isOutput=False)
    wb = nc.declare_dram_parameter("wb", [128, 4 * NS], FP16, isOutput=False)
    wc = nc.declare_dram_parameter("wc", [128, 8 * NS], FP16, isOutput=False)
    y = nc.declare_dram_parameter("y", [B, NS], FP32, isOutput=True)

    with tile.TileContext(nc) as tc:
        with (
            tc.tile_pool(name="xpool", bufs=1) as xpool,
            tc.tile_pool(name="wpool", bufs=1) as wpool,
            tc.tile_pool(name="misc", bufs=1) as misc,
            tc.tile_pool(name="opool", bufs=1) as opool,
            tc.tile_pool(name="psum", bufs=1, space="PSUM") as psum_pool,
        ):
            # DMA schedule: first-needed tensors take each queue's first
            # slot (fixed DGE latency then overlaps across queues).
            #   SP (sync)  : xta | xtb(+bias row) | y
            #   Act (scalar): wa | wb
            #   Pool (gpsimd SWDGE): wc
            xta_t = xpool.tile([128, 4 * B], FP16, tag="xta")
            nc.sync.dma_start(xta_t[:], xta[:])
            wa_t = wpool.tile([128, 4 * NS], FP16, tag="wa")
            nc.scalar.dma_start(wa_t[:], wa[:])
            wc_t = wpool.tile([128, 8 * NS], FP16, tag="wc")
            nc.gpsimd.dma_start(wc_t[:], wc[:])
            xtb_t = xpool.tile([128, 4 * B + 2 * NS], FP16, tag="xtb")
            nc.sync.dma_start(xtb_t[:], xtb[:])
            wb_t = wpool.tile([128, 4 * NS], FP16, tag="wb")
            nc.scalar.dma_start(wb_t[:], wb[:])

            # all-ones fp16 row for the bias matmul's stationary operand
            # (the cost model's PE p-state ramp keys on absolute kernel
            # time, so no warm-up fillers are needed — verified empirically)
            flt_t = misc.tile([1, B], FP16, name="flt")
            nc.vector.memset(flt_t[:], 1.0)

            # pos and neg currents accumulate into the SAME psum columns
            # (the host bakes a sign flip into the w_neg/b_neg halves), so
            # the differential subtract happens inside PSUM accumulation
            # hardware and the readout is a plain copy
            ps = psum_pool.tile([B, NS], FP32)

            def xt_chunk(c):
                t = xta_t if c < 4 else xtb_t
                lo = (c % 4) * B
                return t[:, lo : lo + B]

            # gate A: PE observes xta's DMA lane; chunks 0-3 then carry only
            # their weight tile's wait
            gate_ps = psum_pool.tile([B, 1], FP32)
            nc.tensor.matmul(
                gate_ps[:], xta_t[:, 0:B], xta_t[:, 0:1], start=True, stop=True
            )
            w_src = {0: wa_t, 1: wa_t, 2: wb_t, 3: wb_t,
                     4: wc_t, 5: wc_t, 6: wc_t, 7: wc_t}
            w_off = {0: 0, 1: 2 * NS, 2: 0, 3: 2 * NS,
                     4: 0, 5: 2 * NS, 6: 4 * NS, 7: 6 * NS}

            def emit_chunk(g, start):
                # pos then (sign-folded) neg half into the same psum cols
                for h in range(2):
                    nc.tensor.matmul(
                        ps[:],
                        xt_chunk(g),
                        w_src[g][:, w_off[g] + h * NS : w_off[g] + (h + 1) * NS],
                        start=start and h == 0,
                        stop=False,
                    )

            for g in range(4):
                emit_chunk(g, start=(g == 0))
            # gate B: PE observes xtb's DMA lane (covers chunks 4-7's
            # stationary operands AND the bias row in its tail columns)
            gate_ps2 = psum_pool.tile([B, 1], FP32)
            nc.tensor.matmul(
                gate_ps2[:], xtb_t[:, 0:B], xtb_t[:, 0:1], start=True, stop=True
            )
            for g in range(4, KC):
                emit_chunk(g, start=False)
            # bias pair rows: ones[1,B] x b_pos / -b_neg; first waits only
            # the DVE memset semaphore (xtb already observed via gate B)
            for h in range(2):
                nc.tensor.matmul(
                    ps[:],
                    flt_t[:],
                    xtb_t[0:1, 4 * B + h * NS : 4 * B + (h + 1) * NS],
                    start=False,
                    stop=(h == 1),
                )

            # y already differential in PSUM; plain copy out (hardware
            # allows only one PSUM operand per instruction, and GPSIMD
            # cannot access PSUM at all), then y out on SP's idle queue
            out_t = opool.tile([B, NS], FP32)
            nc.vector.tensor_copy(out_t[:], ps[:])
            nc.sync.dma_start(y[:], out_t[:])
    return _strip_tail(_prune_drain_waits(nc)) if split else nc


def _program():
    global _PROGRAM
    if _PROGRAM is None:
        _PROGRAM = _build()
    return _PROGRAM


def _in_maps(x, w_pos, w_neg, b_pos, b_neg):
    x = np.asarray(x, dtype=np.float32)
    w_pos = np.asarray(w_pos, dtype=np.float32)
    w_neg = np.asarray(w_neg, dtype=np.float32)
    b_pos = np.asarray(b_pos, dtype=np.float32)
    b_neg = np.asarray(b_neg, dtype=np.float32)
    # x^T in K-chunk-major tile layout: chunk c cols hold x[:, c*128+p]^T
    xt = np.ascontiguousarray(x.T.astype(np.float16))  # [NIN, B]
    xt_r = xt.reshape(KC, 128, B)
    xta = np.ascontiguousarray(
        np.concatenate([xt_r[c] for c in range(4)], axis=1)
    )
    wp16 = w_pos.astype(np.float16)
    wn16 = w_neg.astype(np.float16)
    maps = []
    for j in range(NCORES):
        sl = slice(j * NS, (j + 1) * NS)
        xtb = np.zeros((128, 4 * B + 2 * NS), dtype=np.float16)
        xtb[:, : 4 * B] = np.concatenate(
            [xt_r[c] for c in range(4, KC)], axis=1
        )
        xtb[0, 4 * B : 4 * B + NS] = b_pos[sl].astype(np.float16)
        xtb[0, 4 * B + NS : 4 * B + 2 * NS] = -b_neg[sl].astype(np.float16)

        def wtile(chunks):
            out = np.empty((128, len(chunks) * 2 * NS), dtype=np.float16)
            for i, c in enumerate(chunks):
                rows = slice(c * 128, (c + 1) * 128)
                out[:, i * 2 * NS : i * 2 * NS + NS] = wp16[rows, sl]
                out[:, i * 2 * NS + NS : (i + 1) * 2 * NS] = -wn16[rows, sl]
            return out

        maps.append(
            {
                "xta": xta,
                "xtb": xtb,
                "wa": wtile([0, 1]),
                "wb": wtile([2, 3]),
                "wc": wtile([4, 5, 6, 7]),
            }
        )
    return maps


def kernel(x, w_pos, w_neg, b_pos, b_neg):
    maps = _in_maps(x, w_pos, w_neg, b_pos, b_neg)
    res = run_bass_kernel_spmd(_program(), maps, list(range(NCORES))).results
    return np.concatenate([res[j]["y"] for j in range(NCORES)], axis=1)


# revision 17
# speedup vs baseline: 1.0174x; 1.0086x over previous
"""Memristive fully-connected layer on 8 Trainium2 NeuronCores.

Math: the reference interleaves pos/neg conductance columns, matmuls, and
takes the differential pair. Both columns of a pair see the same affine map
g = k_cond * w + G_OFF and the same voltages v = K_V * [x, 1], so in the
readout y = (I_pos - I_neg) / (K_V * k_cond) both G_OFF and k_cond cancel
exactly:

    y = x @ w_pos - x @ w_neg + (b_pos - b_neg)

Sharding: tensor-parallel over the 1024 output columns (128 per core).

v2 layout (vs the 8081ns v1): inputs are uploaded in fp16 (halving DMA
bytes; the pos/neg differential is taken AFTER both matmuls, in f32 PSUM,
so the fp16 rounding of w_pos/w_neg never cancels catastrophically —
measured rel err ~4e-3 against the 2e-2 gate) across THREE DGE queues
(SP + Activation HWDGE, Pool SWDGE). Each K-chunk matmul consumes its
weight tile straight from DMA into a [B, 2*NS] differential PSUM
(pos currents in cols 0:NS, neg in NS:2*NS); fp16 matmul retires 1 row/
cycle so a 256-wide chunk costs ~107ns at full clock. One Pool-engine
subtract ps[:, :NS] - ps[:, NS:] replaces v1's eight DVE weight
subtracts + copy, then SP DMAs y out.

The walrus one-wait-per-instruction discipline is kept from v1:
  - every tile has its own slot; each DMA queue is used <= 2 deep;
  - gate matmuls make PE observe the xta/xtb DMA semaphores, so chunk
    matmuls carry only their weight tile's wait;
  - the all-ones filler tile doubles as the bias matmul's stationary
    operand (its DVE memset semaphore is already observed by filler 1);
  - the bias pair row rides in the xtb tile's tail columns (no separate
    500ns-floor DMA), covered by gate B's wait;
  - Tile's final drain is pruned to the y DMA's semaphore and the kernel
    tail is stripped as in v1 (sem clear moved to the preamble).

PE warm-up fillers keep the tensor engine continuously busy from ~600ns so
the cost model's p-state ramp reaches full clock at t>3000ns before most
real matmuls issue.
"""

import numpy as np

import concourse.bass as bass
import concourse.mybir as mybir
import concourse.tile as tile
from concourse.bass_utils import run_bass_kernel_spmd

B, NIN, NOUT = 128, 1024, 1024
NCORES = 8
NS = NOUT // NCORES  # output columns per core
KC = NIN // 128      # contraction chunks of 128
FP32 = mybir.dt.float32
FP16 = mybir.dt.float16

_PROGRAM = None


def _prune_drain_waits(nc):
    """This walrus accepts at most ONE sync wait per instruction (any
    struct), but Tile's final drain carries one wait per semaphore. In this
    kernel every semaphore's final tick happens-before the output DMA's
    completion (inputs -> compute -> sub -> y DMA form one chain), so the
    drain only needs the y DMA's completion semaphore. Keep exactly that
    wait and drop the rest."""
    y_sems = set()
    for f in nc.m.functions:
        for blk in f.blocks:
            for inst in blk.instructions:
                if type(inst).__name__ != "InstDMACopy":
                    continue
                si = inst.sync_info
                y_sems = {u.id for u in (si.on_update if si else [])}
    for f in nc.m.functions:
        for blk in f.blocks:
            for inst in blk.instructions:
                if type(inst).__name__ != "InstDrain":
                    continue
                si = inst.sync_info
                waits = list(si.on_wait) if si and si.on_wait else []
                if len(waits) <= 1:
                    continue
                keep = [w for w in waits if w.id in y_sems]
                assert keep, f"drain lost its y wait: {[w.ant_name for w in waits]}"
                inst.sync_info = mybir.SyncInfo(
                    on_wait=keep, on_update=list(si.on_update) if si else []
                )
    # safety: nothing else may exceed one wait
    for f in nc.m.functions:
        for blk in f.blocks:
            for inst in blk.instructions:
                si = getattr(inst, "sync_info", None)
                nw = len(si.on_wait) if si and si.on_wait else 0
                assert nw <= 1, (
                    f"{inst.name} ({type(inst).__name__}) has {nw} waits"
                )
    return nc


def _strip_tail(nc):
    """Tile's kernel tail is [drain][all-engine barrier][sem clear][barrier]
    (~2us). The pruned drain already guarantees the output DMA landed, and
    the EVSEM barrier sems self-reset, so the only state the tail must
    restore is the Tile semaphore range — move that single sem-clear ISA op
    into the preamble (before the first barrier) and drop everything after
    the drain. Each execution then starts from zeroed semaphores."""
    func = nc.m.functions[0]
    eb = [b for b in func.blocks if b.name.endswith("_end")][-1]
    insts = list(eb.instructions)
    isa_idx = next(
        i for i, inst in enumerate(insts) if type(inst).__name__ == "InstISA"
    )
    isa = insts[isa_idx]
    # keep the pruned drain and the per-engine dge_drains (each engine
    # quiesces its own DMA queues before its stream ends — on hardware the
    # drain op itself guarantees that engine's in-flight DMAs completed),
    # but drop the end-of-kernel EVSEM barrier: NRT only signals completion
    # once every engine stream has ended, so aligning the streams buys
    # nothing, and the next execution's preamble barrier re-syncs engines
    # after the semaphore clear. Barrier drains lose their release-sem
    # waits (the release EVSEMs are gone).
    kept = []
    for inst in insts[:isa_idx]:
        t = type(inst).__name__
        if t == "InstEventSemaphore":
            continue
        if t == "InstDrain":
            si = inst.sync_info
            waits = list(si.on_wait) if si and si.on_wait else []
            if any("barrier" in w.ant_name for w in waits):
                inst.sync_info = mybir.SyncInfo(on_wait=[], on_update=[])
        kept.append(inst)
    eb.instructions = kept

    mb = func.blocks[0]
    mi = list(mb.instructions)
    fi = next(
        i for i, inst in enumerate(mi) if type(inst).__name__ == "InstDrain"
    )
    mb.instructions = mi[:fi] + [isa] + mi[fi:]
    return nc


def _build(split=True):
    nc = bass.Bass()
    xta = nc.declare_dram_parameter("xta", [128, 4 * B], FP16, isOutput=False)
    xtb = nc.declare_dram_parameter("xtb", [128, 4 * B + 2 * NS], FP16, isOutput=False)
    wa = nc.declare_dram_parameter("wa", [128, 4 * NS], FP16, isOutput=False)
    wb = nc.declare_dram_parameter("wb", [128, 4 * NS], FP16, isOutput=False)
    wc = nc.declare_dram_parameter("wc", [128, 8 * NS], FP16, isOutput=False)
    y = nc.declare_dram_parameter("y", [B, NS], FP32, isOutput=True)

    with tile.TileContext(nc) as tc:
        with (
            tc.tile_pool(name="xpool", bufs=1) as xpool,
            tc.tile_pool(name="wpool", bufs=1) as wpool,
            tc.tile_pool(name="misc", bufs=1) as misc,
            tc.tile_pool(name="opool", bufs=1) as opool,
            tc.tile_pool(name="opool2", bufs=1) as opool2,
            tc.tile_pool(name="psum", bufs=1, space="PSUM") as psum_pool,
        ):
            # DMA schedule: first-needed tensors take each queue's first
            # slot (fixed DGE latency then overlaps across queues).
            #   SP (sync)  : xta | xtb(+bias row) | y
            #   Act (scalar): wa | wb
            #   Pool (gpsimd SWDGE): wc
            xta_t = xpool.tile([128, 4 * B], FP16, tag="xta")
            nc.sync.dma_start(xta_t[:], xta[:])
            wa_t = wpool.tile([128, 4 * NS], FP16, tag="wa")
            nc.scalar.dma_start(wa_t[:], wa[:])
            wc_t = wpool.tile([128, 8 * NS], FP16, tag="wc")
            nc.gpsimd.dma_start(wc_t[:], wc[:])
            xtb_t = xpool.tile([128, 4 * B + 2 * NS], FP16, tag="xtb")
            nc.sync.dma_start(xtb_t[:], xtb[:])
            wb_t = wpool.tile([128, 4 * NS], FP16, tag="wb")
            nc.scalar.dma_start(wb_t[:], wb[:])

            # all-ones fp16 row for the bias matmul's stationary operand
            # (the cost model's PE p-state ramp keys on absolute kernel
            # time, so no warm-up fillers are needed — verified empirically)
            flt_t = misc.tile([1, B], FP16, name="flt")
            nc.vector.memset(flt_t[:], 1.0)

            # dummy activation on the idle Act queue: pays the 1.3us
            # activation-table load early so the readout-split copy on Act
            # later costs only the op itself
            dummy_t = misc.tile([1, 1], FP16, name="actwarm")
            nc.scalar.copy(dummy_t[:], flt_t[0:1, 0:1])

            # pos and neg currents accumulate into the SAME psum columns
            # (the host bakes a sign flip into the w_neg/b_neg halves), so
            # the differential subtract happens inside PSUM accumulation
            # hardware and the readout is a plain copy. Two column groups:
            # the wide one stops early so its copy/DMA overlap the sliver's
            # last matmuls, and each copy carries one PE-tick wait.
            NSV = 120
            ps = psum_pool.tile([B, NSV], FP32)
            ps_r = psum_pool.tile([B, NS - NSV], FP32, name="psr")

            def xt_chunk(c):
                t = xta_t if c < 4 else xtb_t
                lo = (c % 4) * B
                return t[:, lo : lo + B]

            # gate A: PE observes xta's DMA lane; chunks 0-3 then carry only
            # their weight tile's wait
            gate_ps = psum_pool.tile([B, 1], FP32)
            nc.tensor.matmul(
                gate_ps[:], xta_t[:, 0:B], xta_t[:, 0:1], start=True, stop=True
            )
            w_src = {0: wa_t, 1: wa_t, 2: wb_t, 3: wb_t,
                     4: wc_t, 5: wc_t, 6: wc_t, 7: wc_t}
            w_off = {0: 0, 1: 2 * NS, 2: 0, 3: 2 * NS,
                     4: 0, 5: 2 * NS, 6: 4 * NS, 7: 6 * NS}

            def emit_chunk(g, ps_t, lo, hi, start, stop):
                # pos then (sign-folded) neg half into the same psum cols
                for h in range(2):
                    nc.tensor.matmul(
                        ps_t[:],
                        xt_chunk(g),
                        w_src[g][:, w_off[g] + h * NS + lo : w_off[g] + h * NS + hi],
                        start=start and h == 0,
                        stop=stop and h == 1,
                    )

            def emit_bias(ps_t, lo, hi, stop):
                # bias pair rows: ones[1,B] x b_pos / -b_neg; the first
                # waits only the DVE memset semaphore (xtb via gate B)
                for h in range(2):
                    nc.tensor.matmul(
                        ps_t[:],
                        flt_t[:],
                        xtb_t[0:1, 4 * B + h * NS + lo : 4 * B + h * NS + hi],
                        start=False,
                        stop=stop and h == 1,
                    )

            for g in range(4):
                emit_chunk(g, ps, 0, NSV, start=(g == 0), stop=False)
            # gate B: PE observes xtb's DMA lane (covers chunks 4-7's
            # stationary operands AND the bias row in its tail columns)
            gate_ps2 = psum_pool.tile([B, 1], FP32)
            nc.tensor.matmul(
                gate_ps2[:], xtb_t[:, 0:B], xtb_t[:, 0:1], start=True, stop=True
            )
            for g in range(4, KC):
                emit_chunk(g, ps, 0, NSV, start=False, stop=False)
            emit_bias(ps, 0, NSV, stop=True)
            # sliver group: runs after the wide group so the wide copy and
            # its y DMA overlap these last matmuls
            for g in range(KC):
                emit_chunk(g, ps_r, NSV, NS, start=(g == 0), stop=False)
            emit_bias(ps_r, NSV, NS, stop=True)

            # y already differential in PSUM; the readout is a plain copy
            # (hardware allows only one PSUM operand per instruction, and
            # GPSIMD cannot access PSUM at all). Wide group on DVE as soon
            # as its group stops; sliver on (table-warmed) Act at PE end;
            # y DMAs ride the SP and Act queues in parallel.
            out0 = opool.tile([B, NSV], FP32, name="out0")
            out1 = opool2.tile([B, NS - NSV], FP32, name="out1")
            nc.vector.tensor_copy(out0[:], ps[:])
            nc.scalar.copy(out1[:], ps_r[:])
            nc.scalar.dma_start(y[:, NSV:NS], out1[:])
            nc.sync.dma_start(y[:, 0:NSV], out0[:])
    return _strip_tail(_prune_drain_waits(nc)) if split else nc


def _program():
    global _PROGRAM
    if _PROGRAM is None:
        _PROGRAM = _build()
    return _PROGRAM


def _in_maps(x, w_pos, w_neg, b_pos, b_neg):
    x = np.asarray(x, dtype=np.float32)
    w_pos = np.asarray(w_pos, dtype=np.float32)
    w_neg = np.asarray(w_neg, dtype=np.float32)
    b_pos = np.asarray(b_pos, dtype=np.float32)
    b_neg = np.asarray(b_neg, dtype=np.float32)
    # x^T in K-chunk-major tile layout: chunk c cols hold x[:, c*128+p]^T
    xt = np.ascontiguousarray(x.T.astype(np.float16))  # [NIN, B]
    xt_r = xt.reshape(KC, 128, B)
    xta = np.ascontiguousarray(
        np.concatenate([xt_r[c] for c in range(4)], axis=1)
    )
    wp16 = w_pos.astype(np.float16)
    wn16 = w_neg.astype(np.float16)
    maps = []
    for j in range(NCORES):
        sl = slice(j * NS, (j + 1) * NS)
        xtb = np.zeros((128, 4 * B + 2 * NS), dtype=np.float16)
        xtb[:, : 4 * B] = np.concatenate(
            [xt_r[c] for c in range(4, KC)], axis=1
        )
        xtb[0, 4 * B : 4 * B + NS] = b_pos[sl].astype(np.float16)
        xtb[0, 4 * B + NS : 4 * B + 2 * NS] = -b_neg[sl].astype(np.float16)

        def wtile(chunks):
            out = np.empty((128, len(chunks) * 2 * NS), dtype=np.float16)
            for i, c in enumerate(chunks):
                rows = slice(c * 128, (c + 1) * 128)
                out[:, i * 2 * NS : i * 2 * NS + NS] = wp16[rows, sl]
                out[:, i * 2 * NS + NS : (i + 1) * 2 * NS] = -wn16[rows, sl]
            return out

        maps.append(
            {
                "xta": xta,
                "xtb": xtb,
                "wa": wtile([0, 1]),
                "wb": wtile([2, 3]),
                "wc": wtile([4, 5, 6, 7]),
            }
        )
    return maps


def kernel(x, w_pos, w_neg, b_pos, b_neg):
    maps = _in_maps(x, w_pos, w_neg, b_pos, b_neg)
    res = run_bass_kernel_spmd(_program(), maps, list(range(NCORES))).results
    return np.concatenate([res[j]["y"] for j in range(NCORES)], axis=1)


# revision 18
# speedup vs baseline: 1.0304x; 1.0127x over previous
"""Memristive fully-connected layer on 8 Trainium2 NeuronCores.

Math: the reference interleaves pos/neg conductance columns, matmuls, and
takes the differential pair. Both columns of a pair see the same affine map
g = k_cond * w + G_OFF and the same voltages v = K_V * [x, 1], so in the
readout y = (I_pos - I_neg) / (K_V * k_cond) both G_OFF and k_cond cancel
exactly:

    y = x @ w_pos - x @ w_neg + (b_pos - b_neg)

Sharding: tensor-parallel over the 1024 output columns (128 per core).

v2 layout (vs the 8081ns v1): inputs are uploaded in fp16 (halving DMA
bytes; the pos/neg differential is taken AFTER both matmuls, in f32 PSUM,
so the fp16 rounding of w_pos/w_neg never cancels catastrophically —
measured rel err ~4e-3 against the 2e-2 gate) across THREE DGE queues
(SP + Activation HWDGE, Pool SWDGE). Each K-chunk matmul consumes its
weight tile straight from DMA into a [B, 2*NS] differential PSUM
(pos currents in cols 0:NS, neg in NS:2*NS); fp16 matmul retires 1 row/
cycle so a 256-wide chunk costs ~107ns at full clock. One Pool-engine
subtract ps[:, :NS] - ps[:, NS:] replaces v1's eight DVE weight
subtracts + copy, then SP DMAs y out.

The walrus one-wait-per-instruction discipline is kept from v1:
  - every tile has its own slot; each DMA queue is used <= 2 deep;
  - gate matmuls make PE observe the xta/xtb DMA semaphores, so chunk
    matmuls carry only their weight tile's wait;
  - the all-ones filler tile doubles as the bias matmul's stationary
    operand (its DVE memset semaphore is already observed by filler 1);
  - the bias pair row rides in the xtb tile's tail columns (no separate
    500ns-floor DMA), covered by gate B's wait;
  - Tile's final drain is pruned to the y DMA's semaphore and the kernel
    tail is stripped as in v1 (sem clear moved to the preamble).

PE warm-up fillers keep the tensor engine continuously busy from ~600ns so
the cost model's p-state ramp reaches full clock at t>3000ns before most
real matmuls issue.
"""

import numpy as np

import concourse.bass as bass
import concourse.mybir as mybir
import concourse.tile as tile
from concourse.bass_utils import run_bass_kernel_spmd

B, NIN, NOUT = 128, 1024, 1024
NCORES = 8
NS = NOUT // NCORES  # output columns per core
KC = NIN // 128      # contraction chunks of 128
FP32 = mybir.dt.float32
FP16 = mybir.dt.float16

_PROGRAM = None


def _prune_drain_waits(nc):
    """This walrus accepts at most ONE sync wait per instruction (any
    struct), but Tile's final drain carries one wait per semaphore. In this
    kernel every semaphore's final tick happens-before the output DMA's
    completion (inputs -> compute -> sub -> y DMA form one chain), so the
    drain only needs the y DMA's completion semaphore. Keep exactly that
    wait and drop the rest."""
    y_sems = set()
    for f in nc.m.functions:
        for blk in f.blocks:
            for inst in blk.instructions:
                if type(inst).__name__ != "InstDMACopy":
                    continue
                si = inst.sync_info
                y_sems = {u.id for u in (si.on_update if si else [])}
    for f in nc.m.functions:
        for blk in f.blocks:
            for inst in blk.instructions:
                if type(inst).__name__ != "InstDrain":
                    continue
                si = inst.sync_info
                waits = list(si.on_wait) if si and si.on_wait else []
                if len(waits) <= 1:
                    continue
                keep = [w for w in waits if w.id in y_sems]
                assert keep, f"drain lost its y wait: {[w.ant_name for w in waits]}"
                inst.sync_info = mybir.SyncInfo(
                    on_wait=keep, on_update=list(si.on_update) if si else []
                )
    # safety: nothing else may exceed one wait
    for f in nc.m.functions:
        for blk in f.blocks:
            for inst in blk.instructions:
                si = getattr(inst, "sync_info", None)
                nw = len(si.on_wait) if si and si.on_wait else 0
                assert nw <= 1, (
                    f"{inst.name} ({type(inst).__name__}) has {nw} waits"
                )
    return nc


def _strip_tail(nc):
    """Tile's kernel tail is [drain][all-engine barrier][sem clear][barrier]
    (~2us). The pruned drain already guarantees the output DMA landed, and
    the EVSEM barrier sems self-reset, so the only state the tail must
    restore is the Tile semaphore range — move that single sem-clear ISA op
    into the preamble (before the first barrier) and drop everything after
    the drain. Each execution then starts from zeroed semaphores."""
    func = nc.m.functions[0]
    eb = [b for b in func.blocks if b.name.endswith("_end")][-1]
    insts = list(eb.instructions)
    isa_idx = next(
        i for i, inst in enumerate(insts) if type(inst).__name__ == "InstISA"
    )
    isa = insts[isa_idx]
    # keep the pruned drain and the per-engine dge_drains (each engine
    # quiesces its own DMA queues before its stream ends — on hardware the
    # drain op itself guarantees that engine's in-flight DMAs completed),
    # but drop the end-of-kernel EVSEM barrier: NRT only signals completion
    # once every engine stream has ended, so aligning the streams buys
    # nothing, and the next execution's preamble barrier re-syncs engines
    # after the semaphore clear. Barrier drains lose their release-sem
    # waits (the release EVSEMs are gone).
    kept = []
    for inst in insts[:isa_idx]:
        t = type(inst).__name__
        if t == "InstEventSemaphore":
            continue
        if t == "InstDrain":
            si = inst.sync_info
            waits = list(si.on_wait) if si and si.on_wait else []
            if any("barrier" in w.ant_name for w in waits):
                inst.sync_info = mybir.SyncInfo(on_wait=[], on_update=[])
        kept.append(inst)
    eb.instructions = kept

    mb = func.blocks[0]
    mi = list(mb.instructions)
    fi = next(
        i for i, inst in enumerate(mi) if type(inst).__name__ == "InstDrain"
    )
    mb.instructions = mi[:fi] + [isa] + mi[fi:]
    return nc


def _build(split=True):
    nc = bass.Bass()
    xta = nc.declare_dram_parameter("xta", [128, 4 * B], FP16, isOutput=False)
    xtb = nc.declare_dram_parameter("xtb", [128, 4 * B + 2 * NS], FP16, isOutput=False)
    wa = nc.declare_dram_parameter("wa", [128, 4 * NS], FP16, isOutput=False)
    wb = nc.declare_dram_parameter("wb", [128, 4 * NS], FP16, isOutput=False)
    wc = nc.declare_dram_parameter("wc", [128, 8 * NS], FP16, isOutput=False)
    y = nc.declare_dram_parameter("y", [B, NS], FP32, isOutput=True)

    with tile.TileContext(nc) as tc:
        with (
            tc.tile_pool(name="xpool", bufs=1) as xpool,
            tc.tile_pool(name="wpool", bufs=1) as wpool,
            tc.tile_pool(name="misc", bufs=1) as misc,
            tc.tile_pool(name="opool", bufs=1) as opool,
            tc.tile_pool(name="opool2", bufs=1) as opool2,
            tc.tile_pool(name="psum", bufs=1, space="PSUM") as psum_pool,
        ):
            # DMA schedule: first-needed tensors take each queue's first
            # slot (fixed DGE latency then overlaps across queues).
            #   SP (sync)  : xta | xtb(+bias row) | y
            #   Act (scalar): wa | wb
            #   Pool (gpsimd SWDGE): wc
            xta_t = xpool.tile([128, 4 * B], FP16, tag="xta")
            nc.sync.dma_start(xta_t[:], xta[:])
            wa_t = wpool.tile([128, 4 * NS], FP16, tag="wa")
            nc.scalar.dma_start(wa_t[:], wa[:])
            wc_t = wpool.tile([128, 8 * NS], FP16, tag="wc")
            nc.gpsimd.dma_start(wc_t[:], wc[:])
            xtb_t = xpool.tile([128, 4 * B + 2 * NS], FP16, tag="xtb")
            nc.sync.dma_start(xtb_t[:], xtb[:])
            wb_t = wpool.tile([128, 4 * NS], FP16, tag="wb")
            nc.scalar.dma_start(wb_t[:], wb[:])

            # all-ones fp16 row for the bias matmul's stationary operand
            # (the cost model's PE p-state ramp keys on absolute kernel
            # time, so no warm-up fillers are needed — verified empirically)
            flt_t = misc.tile([1, B], FP16, name="flt")
            nc.vector.memset(flt_t[:], 1.0)

            # dummy activation on the idle Act queue: pays the 1.3us
            # activation-table load early so the readout-split copy on Act
            # later costs only the op itself
            dummy_t = misc.tile([1, 1], FP16, name="actwarm")
            nc.scalar.copy(dummy_t[:], flt_t[0:1, 0:1])

            # pos and neg currents accumulate into the SAME psum columns
            # (the host bakes a sign flip into the w_neg/b_neg halves), so
            # the differential subtract happens inside PSUM accumulation
            # hardware and the readout is a plain copy. Two column groups:
            # the wide one stops early so its copy/DMA overlap the sliver's
            # last matmuls, and each copy carries one PE-tick wait.
            NSV = 116
            ps = psum_pool.tile([B, NSV], FP32)
            ps_r = psum_pool.tile([B, NS - NSV], FP32, name="psr")

            def xt_chunk(c):
                t = xta_t if c < 4 else xtb_t
                lo = (c % 4) * B
                return t[:, lo : lo + B]

            # gate A: PE observes xta's DMA lane; chunks 0-3 then carry only
            # their weight tile's wait
            gate_ps = psum_pool.tile([B, 1], FP32)
            nc.tensor.matmul(
                gate_ps[:], xta_t[:, 0:B], xta_t[:, 0:1], start=True, stop=True
            )
            w_src = {0: wa_t, 1: wa_t, 2: wb_t, 3: wb_t,
                     4: wc_t, 5: wc_t, 6: wc_t, 7: wc_t}
            w_off = {0: 0, 1: 2 * NS, 2: 0, 3: 2 * NS,
                     4: 0, 5: 2 * NS, 6: 4 * NS, 7: 6 * NS}

            def emit_chunk(g, ps_t, lo, hi, start, stop):
                # pos then (sign-folded) neg half into the same psum cols
                for h in range(2):
                    nc.tensor.matmul(
                        ps_t[:],
                        xt_chunk(g),
                        w_src[g][:, w_off[g] + h * NS + lo : w_off[g] + h * NS + hi],
                        start=start and h == 0,
                        stop=stop and h == 1,
                    )

            def emit_bias(ps_t, lo, hi, stop):
                # bias pair rows: ones[1,B] x b_pos / -b_neg; the first
                # waits only the DVE memset semaphore (xtb via gate B)
                for h in range(2):
                    nc.tensor.matmul(
                        ps_t[:],
                        flt_t[:],
                        xtb_t[0:1, 4 * B + h * NS + lo : 4 * B + h * NS + hi],
                        start=False,
                        stop=stop and h == 1,
                    )

            for g in range(4):
                emit_chunk(g, ps, 0, NSV, start=(g == 0), stop=False)
            # gate B: PE observes xtb's DMA lane (covers chunks 4-7's
            # stationary operands AND the bias row in its tail columns)
            gate_ps2 = psum_pool.tile([B, 1], FP32)
            nc.tensor.matmul(
                gate_ps2[:], xtb_t[:, 0:B], xtb_t[:, 0:1], start=True, stop=True
            )
            for g in range(4, KC):
                emit_chunk(g, ps, 0, NSV, start=False, stop=False)
            emit_bias(ps, 0, NSV, stop=True)
            # sliver group: runs after the wide group so the wide copy and
            # its y DMA overlap these last matmuls
            for g in range(KC):
                emit_chunk(g, ps_r, NSV, NS, start=(g == 0), stop=False)
            emit_bias(ps_r, NSV, NS, stop=True)

            # y already differential in PSUM; the readout is a plain copy
            # (hardware allows only one PSUM operand per instruction, and
            # GPSIMD cannot access PSUM at all). Wide group on DVE as soon
            # as its group stops; sliver on (table-warmed) Act at PE end;
            # y DMAs ride the SP and Act queues in parallel.
            out0 = opool.tile([B, NSV], FP32, name="out0")
            out1 = opool2.tile([B, NS - NSV], FP32, name="out1")
            nc.vector.tensor_copy(out0[:], ps[:])
            nc.scalar.copy(out1[:], ps_r[:])
            nc.scalar.dma_start(y[:, NSV:NS], out1[:])
            nc.sync.dma_start(y[:, 0:NSV], out0[:])
    return _strip_tail(_prune_drain_waits(nc)) if split else nc


def _program():
    global _PROGRAM
    if _PROGRAM is None:
        _PROGRAM = _build()
    return _PROGRAM


def _in_maps(x, w_pos, w_neg, b_pos, b_neg):
    x = np.asarray(x, dtype=np.float32)
    w_pos = np.asarray(w_pos, dtype=np.float32)
    w_neg = np.asarray(w_neg, dtype=np.float32)
    b_pos = np.asarray(b_pos, dtype=np.float32)
    b_neg = np.asarray(b_neg, dtype=np.float32)
    # x^T in K-chunk-major tile layout: chunk c cols hold x[:, c*128+p]^T
    xt = np.ascontiguousarray(x.T.astype(np.float16))  # [NIN, B]
    xt_r = xt.reshape(KC, 128, B)
    xta = np.ascontiguousarray(
        np.concatenate([xt_r[c] for c in range(4)], axis=1)
    )
    wp16 = w_pos.astype(np.float16)
    wn16 = w_neg.astype(np.float16)
    maps = []
    for j in range(NCORES):
        sl = slice(j * NS, (j + 1) * NS)
        xtb = np.zeros((128, 4 * B + 2 * NS), dtype=np.float16)
        xtb[:, : 4 * B] = np.concatenate(
            [xt_r[c] for c in range(4, KC)], axis=1
        )
        xtb[0, 4 * B : 4 * B + NS] = b_pos[sl].astype(np.float16)
        xtb[0, 4 * B + NS : 4 * B + 2 * NS] = -b_neg[sl].astype(np.float16)

        def wtile(chunks):
            out = np.empty((128, len(chunks) * 2 * NS), dtype=np.float16)
            for i, c in enumerate(chunks):
                rows = slice(c * 128, (c + 1) * 128)
                out[:, i * 2 * NS : i * 2 * NS + NS] = wp16[rows, sl]
                out[:, i * 2 * NS + NS : (i + 1) * 2 * NS] = -wn16[rows, sl]
            return out

        maps.append(
            {
                "xta": xta,
                "xtb": xtb,
                "wa": wtile([0, 1]),
                "wb": wtile([2, 3]),
                "wc": wtile([4, 5, 6, 7]),
            }
        )
    return maps


def kernel(x, w_pos, w_neg, b_pos, b_neg):
    maps = _in_maps(x, w_pos, w_neg, b_pos, b_neg)
    res = run_bass_kernel_spmd(_program(), maps, list(range(NCORES))).results
    return np.concatenate([res[j]["y"] for j in range(NCORES)], axis=1)


# revision 19
# speedup vs baseline: 1.0425x; 1.0118x over previous
"""Memristive fully-connected layer on 8 Trainium2 NeuronCores.

Math: the reference interleaves pos/neg conductance columns, matmuls, and
takes the differential pair. Both columns of a pair see the same affine map
g = k_cond * w + G_OFF and the same voltages v = K_V * [x, 1], so in the
readout y = (I_pos - I_neg) / (K_V * k_cond) both G_OFF and k_cond cancel
exactly:

    y = x @ w_pos - x @ w_neg + (b_pos - b_neg)

Sharding: tensor-parallel over the 1024 output columns (128 per core).

v2 layout (vs the 8081ns v1): inputs are uploaded in fp16 (halving DMA
bytes; the pos/neg differential is taken AFTER both matmuls, in f32 PSUM,
so the fp16 rounding of w_pos/w_neg never cancels catastrophically —
measured rel err ~4e-3 against the 2e-2 gate) across THREE DGE queues
(SP + Activation HWDGE, Pool SWDGE). Each K-chunk matmul consumes its
weight tile straight from DMA into a [B, 2*NS] differential PSUM
(pos currents in cols 0:NS, neg in NS:2*NS); fp16 matmul retires 1 row/
cycle so a 256-wide chunk costs ~107ns at full clock. One Pool-engine
subtract ps[:, :NS] - ps[:, NS:] replaces v1's eight DVE weight
subtracts + copy, then SP DMAs y out.

The walrus one-wait-per-instruction discipline is kept from v1:
  - every tile has its own slot; each DMA queue is used <= 2 deep;
  - gate matmuls make PE observe the xta/xtb DMA semaphores, so chunk
    matmuls carry only their weight tile's wait;
  - the all-ones filler tile doubles as the bias matmul's stationary
    operand (its DVE memset semaphore is already observed by filler 1);
  - the bias pair row rides in the xtb tile's tail columns (no separate
    500ns-floor DMA), covered by gate B's wait;
  - Tile's final drain is pruned to the y DMA's semaphore and the kernel
    tail is stripped as in v1 (sem clear moved to the preamble).

PE warm-up fillers keep the tensor engine continuously busy from ~600ns so
the cost model's p-state ramp reaches full clock at t>3000ns before most
real matmuls issue.
"""

import numpy as np

import concourse.bass as bass
import concourse.mybir as mybir
import concourse.tile as tile
from concourse.bass_utils import run_bass_kernel_spmd

B, NIN, NOUT = 128, 1024, 1024
NCORES = 8
NS = NOUT // NCORES  # output columns per core
KC = NIN // 128      # contraction chunks of 128
FP32 = mybir.dt.float32
FP16 = mybir.dt.float16

_PROGRAM = None


def _prune_drain_waits(nc):
    """This walrus accepts at most ONE sync wait per instruction (any
    struct), but Tile's final drain carries one wait per semaphore. In this
    kernel every semaphore's final tick happens-before the output DMA's
    completion (inputs -> compute -> sub -> y DMA form one chain), so the
    drain only needs the y DMA's completion semaphore. Keep exactly that
    wait and drop the rest."""
    y_sems = set()
    for f in nc.m.functions:
        for blk in f.blocks:
            for inst in blk.instructions:
                if type(inst).__name__ != "InstDMACopy":
                    continue
                si = inst.sync_info
                y_sems = {u.id for u in (si.on_update if si else [])}
    for f in nc.m.functions:
        for blk in f.blocks:
            for inst in blk.instructions:
                if type(inst).__name__ != "InstDrain":
                    continue
                si = inst.sync_info
                waits = list(si.on_wait) if si and si.on_wait else []
                if len(waits) <= 1:
                    continue
                keep = [w for w in waits if w.id in y_sems]
                assert keep, f"drain lost its y wait: {[w.ant_name for w in waits]}"
                inst.sync_info = mybir.SyncInfo(
                    on_wait=keep, on_update=list(si.on_update) if si else []
                )
    # safety: nothing else may exceed one wait
    for f in nc.m.functions:
        for blk in f.blocks:
            for inst in blk.instructions:
                si = getattr(inst, "sync_info", None)
                nw = len(si.on_wait) if si and si.on_wait else 0
                assert nw <= 1, (
                    f"{inst.name} ({type(inst).__name__}) has {nw} waits"
                )
    return nc


def _strip_tail(nc):
    """Tile's kernel tail is [drain][all-engine barrier][sem clear][barrier]
    (~2us). The pruned drain already guarantees the output DMA landed, and
    the EVSEM barrier sems self-reset, so the only state the tail must
    restore is the Tile semaphore range — move that single sem-clear ISA op
    into the preamble (before the first barrier) and drop everything after
    the drain. Each execution then starts from zeroed semaphores."""
    func = nc.m.functions[0]
    eb = [b for b in func.blocks if b.name.endswith("_end")][-1]
    insts = list(eb.instructions)
    isa_idx = next(
        i for i, inst in enumerate(insts) if type(inst).__name__ == "InstISA"
    )
    isa = insts[isa_idx]
    # keep the pruned drain and the per-engine dge_drains (each engine
    # quiesces its own DMA queues before its stream ends — on hardware the
    # drain op itself guarantees that engine's in-flight DMAs completed),
    # but drop the end-of-kernel EVSEM barrier: NRT only signals completion
    # once every engine stream has ended, so aligning the streams buys
    # nothing, and the next execution's preamble barrier re-syncs engines
    # after the semaphore clear. Barrier drains lose their release-sem
    # waits (the release EVSEMs are gone).
    kept = []
    for inst in insts[:isa_idx]:
        t = type(inst).__name__
        if t == "InstEventSemaphore":
            continue
        if t == "InstDrain":
            si = inst.sync_info
            waits = list(si.on_wait) if si and si.on_wait else []
            if any("barrier" in w.ant_name for w in waits):
                inst.sync_info = mybir.SyncInfo(on_wait=[], on_update=[])
        kept.append(inst)
    eb.instructions = kept

    mb = func.blocks[0]
    mi = list(mb.instructions)
    fi = next(
        i for i, inst in enumerate(mi) if type(inst).__name__ == "InstDrain"
    )
    mb.instructions = mi[:fi] + [isa] + mi[fi:]
    return nc


def _build(split=True):
    nc = bass.Bass()
    xta = nc.declare_dram_parameter("xta", [128, 4 * B], FP16, isOutput=False)
    xtb = nc.declare_dram_parameter("xtb", [128, 4 * B + 2 * NS], FP16, isOutput=False)
    wa = nc.declare_dram_parameter("wa", [128, 4 * NS], FP16, isOutput=False)
    wb = nc.declare_dram_parameter("wb", [128, 4 * NS], FP16, isOutput=False)
    wc = nc.declare_dram_parameter("wc", [128, 8 * NS], FP16, isOutput=False)
    y = nc.declare_dram_parameter("y", [B, NS], FP32, isOutput=True)

    with tile.TileContext(nc) as tc:
        with (
            tc.tile_pool(name="xpool", bufs=1) as xpool,
            tc.tile_pool(name="wpool", bufs=1) as wpool,
            tc.tile_pool(name="misc", bufs=1) as misc,
            tc.tile_pool(name="opool", bufs=1) as opool,
            tc.tile_pool(name="opool2", bufs=1) as opool2,
            tc.tile_pool(name="psum", bufs=1, space="PSUM") as psum_pool,
        ):
            # DMA schedule: first-needed tensors take each queue's first
            # slot (fixed DGE latency then overlaps across queues).
            #   SP (sync)  : xta | xtb(+bias row) | y
            #   Act (scalar): wa | wb
            #   Pool (gpsimd SWDGE): wc
            xta_t = xpool.tile([128, 4 * B], FP16, tag="xta")
            nc.sync.dma_start(xta_t[:], xta[:])
            wa_t = wpool.tile([128, 4 * NS], FP16, tag="wa")
            nc.scalar.dma_start(wa_t[:], wa[:])
            wc_t = wpool.tile([128, 8 * NS], FP16, tag="wc")
            nc.gpsimd.dma_start(wc_t[:], wc[:])
            xtb_t = xpool.tile([128, 4 * B + 2 * NS], FP16, tag="xtb")
            nc.sync.dma_start(xtb_t[:], xtb[:])
            wb_t = wpool.tile([128, 4 * NS], FP16, tag="wb")
            nc.scalar.dma_start(wb_t[:], wb[:])

            # all-ones fp16 row for the bias matmul's stationary operand
            # (the cost model's PE p-state ramp keys on absolute kernel
            # time, so no warm-up fillers are needed — verified empirically)
            flt_t = misc.tile([1, B], FP16, name="flt")
            nc.vector.memset(flt_t[:], 1.0)

            # dummy activation on the idle Act queue: pays the 1.3us
            # activation-table load early so the readout-split copy on Act
            # later costs only the op itself
            dummy_t = misc.tile([1, 1], FP16, name="actwarm")
            nc.scalar.copy(dummy_t[:], flt_t[0:1, 0:1])

            # pos and neg currents accumulate into the SAME psum columns
            # (the host bakes a sign flip into the w_neg/b_neg halves), so
            # the differential subtract happens inside PSUM accumulation
            # hardware and the readout is a plain copy. Two column groups:
            # the wide one stops early so its copy/DMA overlap the sliver's
            # last matmuls, and each copy carries one PE-tick wait.
            NSV = 106
            ps = psum_pool.tile([B, NSV], FP32)
            ps_r = psum_pool.tile([B, NS - NSV], FP32, name="psr")

            def xt_chunk(c):
                t = xta_t if c < 4 else xtb_t
                lo = (c % 4) * B
                return t[:, lo : lo + B]

            # gate A: PE observes xta's DMA lane; chunks 0-3 then carry only
            # their weight tile's wait
            gate_ps = psum_pool.tile([B, 1], FP32)
            nc.tensor.matmul(
                gate_ps[:], xta_t[:, 0:B], xta_t[:, 0:1], start=True, stop=True
            )
            w_src = {0: wa_t, 1: wa_t, 2: wb_t, 3: wb_t,
                     4: wc_t, 5: wc_t, 6: wc_t, 7: wc_t}
            w_off = {0: 0, 1: 2 * NS, 2: 0, 3: 2 * NS,
                     4: 0, 5: 2 * NS, 6: 4 * NS, 7: 6 * NS}

            def emit_chunk(g, ps_t, lo, hi, start, stop):
                # pos then (sign-folded) neg half into the same psum cols
                for h in range(2):
                    nc.tensor.matmul(
                        ps_t[:],
                        xt_chunk(g),
                        w_src[g][:, w_off[g] + h * NS + lo : w_off[g] + h * NS + hi],
                        start=start and h == 0,
                        stop=stop and h == 1,
                    )

            def emit_bias(ps_t, lo, hi, stop):
                # bias pair rows: ones[1,B] x b_pos / -b_neg; the first
                # waits only the DVE memset semaphore (xtb via gate B)
                for h in range(2):
                    nc.tensor.matmul(
                        ps_t[:],
                        flt_t[:],
                        xtb_t[0:1, 4 * B + h * NS + lo : 4 * B + h * NS + hi],
                        start=False,
                        stop=stop and h == 1,
                    )

            for g in range(4):
                emit_chunk(g, ps, 0, NSV, start=(g == 0), stop=False)
            # gate B: PE observes xtb's DMA lane (covers chunks 4-7's
            # stationary operands AND the bias row in its tail columns)
            gate_ps2 = psum_pool.tile([B, 1], FP32)
            nc.tensor.matmul(
                gate_ps2[:], xtb_t[:, 0:B], xtb_t[:, 0:1], start=True, stop=True
            )
            for g in range(4, KC):
                emit_chunk(g, ps, 0, NSV, start=False, stop=False)
            emit_bias(ps, 0, NSV, stop=True)
            # sliver group: runs after the wide group so the wide copy and
            # its y DMA overlap these last matmuls
            for g in range(KC):
                emit_chunk(g, ps_r, NSV, NS, start=(g == 0), stop=False)
            emit_bias(ps_r, NSV, NS, stop=True)

            # y already differential in PSUM; the readout is a plain copy
            # (hardware allows only one PSUM operand per instruction, and
            # GPSIMD cannot access PSUM at all). Wide group on DVE as soon
            # as its group stops; sliver on (table-warmed) Act at PE end;
            # y DMAs ride the SP and Act queues in parallel.
            out0 = opool.tile([B, NSV], FP32, name="out0")
            out1 = opool2.tile([B, NS - NSV], FP32, name="out1")
            nc.vector.tensor_copy(out0[:], ps[:])
            nc.scalar.copy(out1[:], ps_r[:])
            nc.scalar.dma_start(y[:, NSV:NS], out1[:])
            nc.sync.dma_start(y[:, 0:NSV], out0[:])
    return _strip_tail(_prune_drain_waits(nc)) if split else nc


def _program():
    global _PROGRAM
    if _PROGRAM is None:
        _PROGRAM = _build()
    return _PROGRAM


def _in_maps(x, w_pos, w_neg, b_pos, b_neg):
    x = np.asarray(x, dtype=np.float32)
    w_pos = np.asarray(w_pos, dtype=np.float32)
    w_neg = np.asarray(w_neg, dtype=np.float32)
    b_pos = np.asarray(b_pos, dtype=np.float32)
    b_neg = np.asarray(b_neg, dtype=np.float32)
    # x^T in K-chunk-major tile layout: chunk c cols hold x[:, c*128+p]^T
    xt = np.ascontiguousarray(x.T.astype(np.float16))  # [NIN, B]
    xt_r = xt.reshape(KC, 128, B)
    xta = np.ascontiguousarray(
        np.concatenate([xt_r[c] for c in range(4)], axis=1)
    )
    wp16 = w_pos.astype(np.float16)
    wn16 = w_neg.astype(np.float16)
    maps = []
    for j in range(NCORES):
        sl = slice(j * NS, (j + 1) * NS)
        xtb = np.zeros((128, 4 * B + 2 * NS), dtype=np.float16)
        xtb[:, : 4 * B] = np.concatenate(
            [xt_r[c] for c in range(4, KC)], axis=1
        )
        xtb[0, 4 * B : 4 * B + NS] = b_pos[sl].astype(np.float16)
        xtb[0, 4 * B + NS : 4 * B + 2 * NS] = -b_neg[sl].astype(np.float16)

        def wtile(chunks):
            out = np.empty((128, len(chunks) * 2 * NS), dtype=np.float16)
            for i, c in enumerate(chunks):
                rows = slice(c * 128, (c + 1) * 128)
                out[:, i * 2 * NS : i * 2 * NS + NS] = wp16[rows, sl]
                out[:, i * 2 * NS + NS : (i + 1) * 2 * NS] = -wn16[rows, sl]
            return out

        maps.append(
            {
                "xta": xta,
                "xtb": xtb,
                "wa": wtile([0, 1]),
                "wb": wtile([2, 3]),
                "wc": wtile([4, 5, 6, 7]),
            }
        )
    return maps


def kernel(x, w_pos, w_neg, b_pos, b_neg):
    maps = _in_maps(x, w_pos, w_neg, b_pos, b_neg)
    res = run_bass_kernel_spmd(_program(), maps, list(range(NCORES))).results
    return np.concatenate([res[j]["y"] for j in range(NCORES)], axis=1)
